# revision 9
# baseline (speedup 1.0000x reference)
"""Cross-attention kernel for 8 Trainium2 NeuronCores.

Sharding: core c => batch b = c//4, head-group g = c%4 (3 of 12 heads, 192 dims).
Each core projects q/k/v for its heads, does softmax attention, and computes a
partial output projection (row-split Wo); host sums the 4 partials per batch.

Key tricks:
  - mask compaction on host: only mask==1 key/value positions are shipped
    (~2048 of 4096), zero-padded to a multiple of 128. Padded rows have
    zeroed v and zeroed ones-column so they contribute 0 to both numerator
    and denominator => exact equivalence with the reference's -1e4 bias.
  - transposed layouts end to end (contraction dim on partitions), so no
    on-device transposes are needed.
  - softmax without max-subtraction (scores*scale ~ N(0,1), exp is safe in
    fp32) and without dividing the SxN score matrix: a ones-column appended
    to v yields the denominator Z per output row; only the 64xN attention
    output is normalized.
  - fp16 operands for all matmuls (fp32 PSUM accumulate).
  - projections are interleaved into the attention j-loops to keep the PE
    stream dense (HAM clock gate stays at 8/8).
"""

import numpy as np

import concourse.bass as bass
import concourse.mybir as mybir
import concourse.tile as tile
from concourse import bacc
from concourse.bass_utils import run_bass_kernel_spmd

H = 12
D = 768
HD = 64
SCALE = HD ** -0.5
NQ = 1024
HL = 3            # heads per core
HWID = HL * HD    # 192 head dims per core
DC = D // 128     # 6 contraction chunks

f16 = mybir.dt.float16
f32 = mybir.dt.float32

_programs = {}


def _build(SP: int):
    NCH = SP // 128
    nc = bacc.Bacc("TRN2", target_bir_lowering=False, debug=False, num_devices=8)

    qT = nc.dram_tensor("qT", [D, NQ], f16, kind="ExternalInput")
    kT = nc.dram_tensor("kT", [D, SP], f16, kind="ExternalInput")
    vT = nc.dram_tensor("vT", [D, SP], f16, kind="ExternalInput")
    mv = nc.dram_tensor("mv", [SP], f16, kind="ExternalInput")
    wqT = nc.dram_tensor("wqT", [D, HWID], f16, kind="ExternalInput")
    wkT = nc.dram_tensor("wkT", [D, HWID], f16, kind="ExternalInput")
    wvT = nc.dram_tensor("wvT", [D, HWID], f16, kind="ExternalInput")
    woT = nc.dram_tensor("woT", [HWID, D], f16, kind="ExternalInput")
    out = nc.dram_tensor("out", [NQ, D], f32, kind="ExternalOutput")

    EXPF = mybir.ActivationFunctionType.Exp
    qT_r = qT.ap().rearrange("(c p) n -> p c n", p=128)
    kT_r = kT.ap().rearrange("(c p) n -> p c n", p=128)
    vT_r = vT.ap().rearrange("(c p) n -> p c n", p=128)

    with tile.TileContext(nc) as tc:
        with (
            tc.tile_pool(name="const", bufs=1) as cpool,
            tc.tile_pool(name="work", bufs=2) as wpool,
            tc.tile_pool(name="expp", bufs=4) as epool,
            tc.tile_pool(name="ps", bufs=3, space="PSUM") as pspool,
            tc.tile_pool(name="psa", bufs=1, space="PSUM") as psapool,
        ):
            # ---- input DMAs, chunked + ordered so compute starts early
            wq_in = cpool.tile([128, DC, HWID], f16)
            nc.sync.dma_start(wq_in[:], wqT.ap().rearrange("(c p) n -> p c n", p=128))
            wk_in = cpool.tile([128, DC, HWID], f16)
            nc.sync.dma_start(wk_in[:], wkT.ap().rearrange("(c p) n -> p c n", p=128))
            qT_in = cpool.tile([128, DC, NQ], f16)
            for d in range(DC):
                nc.sync.dma_start(qT_in[:, d, :], qT_r[:, d, :])
            kT_in = cpool.tile([128, DC, SP], f16)
            for d in range(DC):
                nc.sync.dma_start(kT_in[:, d, :], kT_r[:, d, :])
            wv_in = cpool.tile([128, DC, HWID], f16)
            nc.sync.dma_start(wv_in[:], wvT.ap().rearrange("(c p) n -> p c n", p=128))
            vT_in = cpool.tile([128, DC, SP], f16)
            for d in range(DC):
                nc.sync.dma_start(vT_in[:, d, :], vT_r[:, d, :])
            wo_in = cpool.tile([128, 2, D], f16)
            nc.sync.dma_start(wo_in[:, 0, :], woT[0:128, :])
            nc.sync.dma_start(wo_in[0:64, 1, :], woT[128:HWID, :])
            msk = cpool.tile([128, NCH], f16)
            nc.sync.dma_start(msk[:], mv.ap().rearrange("(c p) -> p c", p=128))

            q0 = cpool.tile([128, NQ], f16)
            q1 = cpool.tile([64, NQ], f16)
            k0 = cpool.tile([128, SP], f16)
            k1 = cpool.tile([64, SP], f16)
            vaug = cpool.tile([128, HL * NCH * 65], f16)
            vaug_r = vaug[:].rearrange("p (h j e) -> p h j e", h=HL, j=NCH)
            a0 = cpool.tile([128, NQ], f16)
            a1 = cpool.tile([64, NQ], f16)

            def proj_qk(w_in, src, dst, mt, sg, sw):
                """One [mw, sw] projection tile: dst[:, sg:sg+sw]."""
                mw = 128 if mt == 0 else 64
                ps = pspool.tile([mw, sw], f32, tag="ps")
                for d in range(DC):
                    for nf in range(0, sw, 512):
                        wf = min(512, sw - nf)
                        nc.tensor.matmul(
                            ps[:, nf:nf + wf],
                            w_in[:, d, mt * 128:mt * 128 + mw],
                            src[:, d, sg + nf:sg + nf + wf],
                            start=(d == 0), stop=(d == DC - 1),
                        )
                nc.vector.tensor_copy(dst[:, sg:sg + sw], ps[:])

            def proj_v(j):
                """v projection for s-chunk j into vaug (all 3 heads)."""
                ps = pspool.tile([128, HWID], f32, tag="ps")
                for d in range(DC):
                    nc.tensor.matmul(
                        ps[:], vT_in[:, d, j * 128:(j + 1) * 128], wv_in[:, d, :],
                        start=(d == 0), stop=(d == DC - 1),
                    )
                nc.vector.tensor_copy(
                    vaug_r[:, :, j, 0:64], ps[:].rearrange("p (h e) -> p h e", h=HL)
                )

            def wo_mms(po, nt, kk):
                """Wo partial-projection matmuls for row-tile nt, K-chunk kk."""
                asrc, kw = ((a0, 128), (a1, 64))[kk]
                for nf in range(0, D, 512):
                    wf = min(512, D - nf)
                    nc.tensor.matmul(
                        po[:, nf:nf + wf],
                        asrc[:, nt * 128:(nt + 1) * 128],
                        wo_in[0:kw, kk, nf:nf + wf],
                        start=(kk == 0), stop=(kk == 1),
                    )

            # mask column of vaug (depends only on msk DMA)
            nc.vector.tensor_copy(
                vaug_r[:, :, :, 64],
                msk[:].rearrange("p (u j) -> p u j", u=1).broadcast_to([128, HL, NCH]),
            )

            # ---- projections for heads h0/h1 (m-tile 0): dense PE warmup
            for sg in range(0, NQ, 1024):
                proj_qk(wq_in, qT_in, q0, 0, sg, min(1024, NQ - sg))
            for sg in range(0, SP, 1024):
                proj_qk(wk_in, kT_in, k0, 0, sg, min(1024, SP - sg))

            # m-tile 1 (head h2) work queue, interleaved into h1's j-loop
            m1_work = []
            for sg in range(0, NQ, 512):
                m1_work.append((wq_in, qT_in, q1, sg, min(512, NQ - sg)))
            for sg in range(0, SP, 512):
                m1_work.append((wk_in, kT_in, k1, sg, min(512, SP - sg)))

            def normalize(at, adst):
                zrow = wpool.tile([1, NQ], f32, tag="zrow")
                nc.vector.tensor_copy(zrow[:], at[64:65, :])
                rz = wpool.tile([1, NQ], f32, tag="rz")
                nc.vector.reciprocal(rz[:], zrow[:])
                rzb = wpool.tile([64, NQ], f32, tag="rzb")
                nc.gpsimd.partition_broadcast(rzb[:], rz[:])
                nc.vector.tensor_mul(adst, at[0:64, :], rzb[:])

            # ---- attention per head, with filler PE work interleaved
            for h in range(HL):
                if h == 0:
                    kh, qh, adst = k0[0:64, :], q0[0:64, :], a0[0:64, :]
                elif h == 1:
                    kh, qh, adst = k0[64:128, :], q0[64:128, :], a0[64:128, :]
                else:
                    kh, qh, adst = k1[:, :], q1[:, :], a1[:, :]
                at = psapool.tile([65, NQ], f32)
                for j in range(NCH):
                    # interleaved filler work keeps the PE stream dense
                    if h == 0:
                        proj_v(j)
                    elif h == 1 and m1_work:
                        w_in, src, dst, sg, sw = m1_work.pop(0)
                        proj_qk(w_in, src, dst, 1, sg, sw)
                    sc = pspool.tile([128, NQ], f32, tag="ps")
                    for nf in range(0, NQ, 512):
                        nc.tensor.matmul(
                            sc[:, nf:nf + 512], kh[:, j * 128:(j + 1) * 128],
                            qh[:, nf:nf + 512], start=True, stop=True,
                        )
                    ex = epool.tile([128, NQ], f16)
                    nc.scalar.activation(ex[:], sc[:], EXPF, scale=SCALE)
                    for nf in range(0, NQ, 512):
                        nc.tensor.matmul(
                            at[:, nf:nf + 512],
                            vaug[:, (h * NCH + j) * 65:(h * NCH + j) * 65 + 65],
                            ex[:, nf:nf + 512],
                            start=(j == 0), stop=(j == NCH - 1),
                        )
                # finish any unconsumed m1 projection work before h2 needs it
                if h == 1:
                    while m1_work:
                        w_in, src, dst, sg, sw = m1_work.pop(0)
                        proj_qk(w_in, src, dst, 1, sg, sw)
                normalize(at, adst)

            # ---- output projection
            for nt in range(NQ // 128):
                po = pspool.tile([128, D], f32, tag="ps")
                wo_mms(po, nt, 0)
                wo_mms(po, nt, 1)
                ob = wpool.tile([128, D], f32, tag="ob")
                nc.vector.tensor_copy(ob[:], po[:])
                nc.sync.dma_start(out[nt * 128:(nt + 1) * 128, :], ob[:])

    nc.compile()
    return nc


def _get_program(SP: int):
    if SP not in _programs:
        _programs[SP] = _build(SP)
    return _programs[SP]


def kernel(query, key, value, mask, Wq, Wk, Wv, Wo, bo):
    query = np.asarray(query, np.float32)
    key = np.asarray(key, np.float32)
    value = np.asarray(value, np.float32)
    mask = np.asarray(mask, np.float32)
    Wq = np.asarray(Wq, np.float32)
    Wk = np.asarray(Wk, np.float32)
    Wv = np.asarray(Wv, np.float32)
    Wo = np.asarray(Wo, np.float32)
    bo = np.asarray(bo, np.float32)

    B, N, _ = query.shape
    idxs = [np.nonzero(mask[b] > 0.5)[0] for b in range(B)]
    se_max = max(len(i) for i in idxs)
    SP = max(((se_max + 127) // 128) * 128, 128)
    nc = _get_program(SP)

    in_maps = []
    for c in range(8):
        b, g = c // 4, c % 4
        hs = g * HWID
        idx = idxs[b]
        ne = len(idx)
        kTc = np.zeros((D, SP), np.float16)
        kTc[:, :ne] = key[b].T[:, idx].astype(np.float16)
        vTc = np.zeros((D, SP), np.float16)
        vTc[:, :ne] = value[b].T[:, idx].astype(np.float16)
        mvec = np.zeros((SP,), np.float16)
        mvec[:ne] = 1.0
        in_maps.append({
            "qT": np.ascontiguousarray(query[b].T.astype(np.float16)),
            "kT": kTc,
            "vT": vTc,
            "mv": mvec,
            "wqT": np.ascontiguousarray(Wq[hs:hs + HWID, :].T.astype(np.float16)),
            "wkT": np.ascontiguousarray(Wk[hs:hs + HWID, :].T.astype(np.float16)),
            "wvT": np.ascontiguousarray(Wv[hs:hs + HWID, :].T.astype(np.float16)),
            "woT": np.ascontiguousarray(Wo[:, hs:hs + HWID].T.astype(np.float16)),
        })

    res = run_bass_kernel_spmd(nc, in_maps, list(range(8))).results
    out = np.zeros((B, N, D), np.float32)
    for b in range(B):
        out[b] = res[4 * b]["out"] + res[4 * b + 1]["out"] \
            + res[4 * b + 2]["out"] + res[4 * b + 3]["out"] + bo
    return out


# revision 10
# speedup vs baseline: 11950.2895x; 11950.2895x over previous
"""Cross-attention kernel for 8 Trainium2 NeuronCores.

Sharding: core c => batch b = c//4, head-group g = c%4 (3 of 12 heads, 192 dims).
Each core projects q/k/v for its heads, does softmax attention, and computes a
partial output projection (row-split Wo); host sums the 4 partials per batch.

Key tricks:
  - mask compaction on host: only mask==1 key/value positions are shipped
    (~2048 of 4096), zero-padded to a multiple of 128. Padded rows have
    zeroed v and zeroed ones-column so they contribute 0 to both numerator
    and denominator => exact equivalence with the reference's -1e4 bias.
  - transposed layouts end to end (contraction dim on partitions): no
    on-device transposes.
  - softmax without max-subtraction (scores*scale ~ N(0,1): exp safe in
    fp32) and without dividing the SxN score matrix: a ones-column appended
    to v yields the denominator Z per output row; only the 64xN attention
    output is normalized.
  - fp16 operands for all matmuls (fp32 PSUM accumulate).
  - q/k/v projections are spread through the attention j-loops as PE filler
    so the PE stream stays dense (HAM clock gate at 8/8), with emission
    software-pipelined (scores j+1 issued before attn j).
"""

import numpy as np

import concourse.bass as bass
import concourse.mybir as mybir
import concourse.tile as tile
from concourse import bacc
from concourse.bass_utils import run_bass_kernel_spmd

H = 12
D = 768
HD = 64
SCALE = HD ** -0.5
NQ = 1024
HL = 3            # heads per core
HWID = HL * HD    # 192 head dims per core
DC = D // 128     # 6 contraction chunks

f16 = mybir.dt.float16
f32 = mybir.dt.float32

_programs = {}


def _build(SP: int):
    NCH = SP // 128
    nc = bacc.Bacc("TRN2", target_bir_lowering=False, debug=False, num_devices=8)

    qT = nc.dram_tensor("qT", [D, NQ], f16, kind="ExternalInput")
    kT = nc.dram_tensor("kT", [D, SP], f16, kind="ExternalInput")
    vT = nc.dram_tensor("vT", [D, SP], f16, kind="ExternalInput")
    mv = nc.dram_tensor("mv", [SP], f16, kind="ExternalInput")
    wqT = nc.dram_tensor("wqT", [D, HWID], f16, kind="ExternalInput")
    wkT = nc.dram_tensor("wkT", [D, HWID], f16, kind="ExternalInput")
    wvT = nc.dram_tensor("wvT", [D, HWID], f16, kind="ExternalInput")
    woT = nc.dram_tensor("woT", [HWID, D], f16, kind="ExternalInput")
    out = nc.dram_tensor("out", [NQ, D], f32, kind="ExternalOutput")

    EXPF = mybir.ActivationFunctionType.Exp
    qT_r = qT.ap().rearrange("(c p) n -> p c n", p=128)
    kT_r = kT.ap().rearrange("(c p) n -> p c n", p=128)
    vT_r = vT.ap().rearrange("(c p) n -> p c n", p=128)

    with tile.TileContext(nc) as tc:
        with (
            tc.tile_pool(name="const", bufs=1) as cpool,
            tc.tile_pool(name="work", bufs=2) as wpool,
            tc.tile_pool(name="expp", bufs=10) as epool,
            tc.tile_pool(name="ps", bufs=2, space="PSUM") as pspool,
            tc.tile_pool(name="psf", bufs=2, space="PSUM") as psfpool,
            tc.tile_pool(name="psa", bufs=1, space="PSUM") as psapool,
        ):
            # ---- input DMAs, chunked + ordered so compute starts early
            wq_in = cpool.tile([128, DC, HWID], f16)
            nc.sync.dma_start(wq_in[:], wqT.ap().rearrange("(c p) n -> p c n", p=128))
            wk_in = cpool.tile([128, DC, HWID], f16)
            nc.sync.dma_start(wk_in[:], wkT.ap().rearrange("(c p) n -> p c n", p=128))
            qT_in = cpool.tile([128, DC, NQ], f16)
            for d in range(DC):
                nc.sync.dma_start(qT_in[:, d, :], qT_r[:, d, :])
            kT_in = cpool.tile([128, DC, SP], f16)
            for d in range(DC):
                nc.sync.dma_start(kT_in[:, d, :], kT_r[:, d, :])
            wv_in = cpool.tile([128, DC, HWID], f16)
            nc.sync.dma_start(wv_in[:], wvT.ap().rearrange("(c p) n -> p c n", p=128))
            vT_in = cpool.tile([128, DC, SP], f16)
            for d in range(DC):
                nc.sync.dma_start(vT_in[:, d, :], vT_r[:, d, :])
            wo_in = cpool.tile([128, 2, D], f16)
            nc.sync.dma_start(wo_in[:, 0, :], woT[0:128, :])
            nc.sync.dma_start(wo_in[0:64, 1, :], woT[128:HWID, :])
            msk = cpool.tile([128, NCH], f16)
            nc.sync.dma_start(msk[:], mv.ap().rearrange("(c p) -> p c", p=128))

            q0 = cpool.tile([128, NQ], f16)
            q1 = cpool.tile([64, NQ], f16)
            k0 = cpool.tile([128, SP], f16)
            k1 = cpool.tile([64, SP], f16)
            vaug = cpool.tile([128, HL * NCH * 65], f16)
            vaug_r = vaug[:].rearrange("p (h j e) -> p h j e", h=HL, j=NCH)
            a0 = cpool.tile([128, NQ], f16)
            a1 = cpool.tile([64, NQ], f16)

            def proj_qk(w_in, src, dst, mt, sg, sw, pool, tag):
                mw = 128 if mt == 0 else 64
                ps = pool.tile([mw, sw], f32, tag=tag)
                for d in range(DC):
                    for nf in range(0, sw, 512):
                        wf = min(512, sw - nf)
                        nc.tensor.matmul(
                            ps[:, nf:nf + wf],
                            w_in[:, d, mt * 128:mt * 128 + mw],
                            src[:, d, sg + nf:sg + nf + wf],
                            start=(d == 0), stop=(d == DC - 1),
                        )
                nc.vector.tensor_copy(dst[:, sg:sg + sw], ps[:])

            def proj_v(j):
                ps = psfpool.tile([128, HWID], f32, tag="fill")
                for d in range(DC):
                    nc.tensor.matmul(
                        ps[:], vT_in[:, d, j * 128:(j + 1) * 128], wv_in[:, d, :],
                        start=(d == 0), stop=(d == DC - 1),
                    )
                nc.vector.tensor_copy(
                    vaug_r[:, :, j, 0:64], ps[:].rearrange("p (h e) -> p h e", h=HL)
                )

            def wo_mms(po, nt, kk):
                asrc, kw = ((a0, 128), (a1, 64))[kk]
                for nf in range(0, D, 512):
                    wf = min(512, D - nf)
                    nc.tensor.matmul(
                        po[:, nf:nf + wf],
                        asrc[:, nt * 128:(nt + 1) * 128],
                        wo_in[0:kw, kk, nf:nf + wf],
                        start=(kk == 0), stop=(kk == 1),
                    )

            def normalize(at, adst):
                zrow = wpool.tile([1, NQ], f32, tag="zrow")
                nc.vector.tensor_copy(zrow[:], at[64:65, :])
                rz = wpool.tile([1, NQ], f32, tag="rz")
                nc.vector.reciprocal(rz[:], zrow[:])
                rzb = wpool.tile([64, NQ], f32, tag="rzb")
                nc.gpsimd.partition_broadcast(rzb[:], rz[:])
                nc.vector.tensor_mul(adst, at[0:64, :], rzb[:])

            # mask column of vaug (depends only on msk DMA)
            nc.vector.tensor_copy(
                vaug_r[:, :, :, 64],
                msk[:].rearrange("p (u j) -> p u j", u=1).broadcast_to([128, HL, NCH]),
            )

            # ---- dense prologue: q/k projections for heads h0/h1 (m-tile 0)
            for sg in range(0, NQ, 1024):
                proj_qk(wq_in, qT_in, q0, 0, sg, min(1024, NQ - sg), pspool, "ps")
            for sg in range(0, SP, 1024):
                proj_qk(wk_in, kT_in, k0, 0, sg, min(1024, SP - sg), pspool, "ps")

            # m-tile-1 (head h2) projections: 512-wide chunks used as filler
            m1_work = []
            for sg in range(0, NQ, 512):
                m1_work.append((wq_in, qT_in, q1, sg, min(512, NQ - sg)))
            for sg in range(0, SP, 512):
                m1_work.append((wk_in, kT_in, k1, sg, min(512, SP - sg)))

            # ---- attention per head (software-pipelined emission)
            heads = (
                (k0[0:64, :], q0[0:64, :], a0[0:64, :]),
                (k0[64:128, :], q0[64:128, :], a0[64:128, :]),
                (k1[:, :], q1[:, :], a1[:, :]),
            )
            pending_norm = None
            for h in range(HL):
                kh, qh, adst = heads[h]
                at = psapool.tile([65, NQ], f32)
                prev_ex = None
                for j in range(NCH):
                    # PE filler to keep the matmul stream dense
                    if h == 0:
                        proj_v(j)
                        if j % 3 == 2 and m1_work:
                            w_in, src, dst, sg, sw = m1_work.pop(0)
                            proj_qk(w_in, src, dst, 1, sg, sw, psfpool, "fill")
                    elif h == 1 and j % 2 == 1 and m1_work:
                        w_in, src, dst, sg, sw = m1_work.pop(0)
                        proj_qk(w_in, src, dst, 1, sg, sw, psfpool, "fill")
                    sc = pspool.tile([128, NQ], f32, tag="ps")
                    for nf in range(0, NQ, 512):
                        nc.tensor.matmul(
                            sc[:, nf:nf + 512], kh[:, j * 128:(j + 1) * 128],
                            qh[:, nf:nf + 512], start=True, stop=True,
                        )
                    ex = epool.tile([128, NQ], f16)
                    nc.scalar.activation(ex[:], sc[:], EXPF, scale=SCALE)
                    if prev_ex is not None:
                        pj, pex = prev_ex
                        for nf in range(0, NQ, 512):
                            nc.tensor.matmul(
                                at[:, nf:nf + 512],
                                vaug[:, (h * NCH + pj) * 65:(h * NCH + pj) * 65 + 65],
                                pex[:, nf:nf + 512],
                                start=(pj == 0), stop=False,
                            )
                    prev_ex = (j, ex)
                    # previous head's normalize, emitted late so its DVE work
                    # lands behind this head's first filler copies
                    if j == 1 and pending_norm is not None:
                        normalize(*pending_norm)
                        pending_norm = None
                    if h == 1 and j == NCH - 1:
                        while m1_work:
                            w_in, src, dst, sg, sw = m1_work.pop(0)
                            proj_qk(w_in, src, dst, 1, sg, sw, psfpool, "fill")
                pj, pex = prev_ex
                for nf in range(0, NQ, 512):
                    nc.tensor.matmul(
                        at[:, nf:nf + 512],
                        vaug[:, (h * NCH + pj) * 65:(h * NCH + pj) * 65 + 65],
                        pex[:, nf:nf + 512],
                        start=(pj == 0), stop=True,
                    )
                pending_norm = (at, adst)

            # ---- output projection, overlapped with the last normalize
            po_prev = None
            po0 = pspool.tile([128, D], f32, tag="ps")
            wo_mms(po0, 0, 0)
            po1 = pspool.tile([128, D], f32, tag="ps")
            wo_mms(po1, 1, 0)
            normalize(*pending_norm)     # h2's normalize (gates only kk=1)
            pend = [(po0, 0), (po1, 1)]
            for nt in range(2, NQ // 128 + 2):
                po, pnt = pend.pop(0)
                wo_mms(po, pnt, 1)
                ob = wpool.tile([128, D], f32, tag="ob")
                nc.vector.tensor_copy(ob[:], po[:])
                nc.sync.dma_start(out[pnt * 128:(pnt + 1) * 128, :], ob[:])
                if nt < NQ // 128:
                    pon = pspool.tile([128, D], f32, tag="ps")
                    wo_mms(pon, nt, 0)
                    pend.append((pon, nt))

    nc.compile()
    return nc


def _get_program(SP: int):
    if SP not in _programs:
        _programs[SP] = _build(SP)
    return _programs[SP]


def kernel(query, key, value, mask, Wq, Wk, Wv, Wo, bo):
    query = np.asarray(query, np.float32)
    key = np.asarray(key, np.float32)
    value = np.asarray(value, np.float32)
    mask = np.asarray(mask, np.float32)
    Wq = np.asarray(Wq, np.float32)
    Wk = np.asarray(Wk, np.float32)
    Wv = np.asarray(Wv, np.float32)
    Wo = np.asarray(Wo, np.float32)
    bo = np.asarray(bo, np.float32)

    B, N, _ = query.shape
    idxs = [np.nonzero(mask[b] > 0.5)[0] for b in range(B)]
    se_max = max(len(i) for i in idxs)
    SP = max(((se_max + 127) // 128) * 128, 128)
    nc = _get_program(SP)

    in_maps = []
    for c in range(8):
        b, g = c // 4, c % 4
        hs = g * HWID
        idx = idxs[b]
        ne = len(idx)
        kTc = np.zeros((D, SP), np.float16)
        kTc[:, :ne] = key[b].T[:, idx].astype(np.float16)
        vTc = np.zeros((D, SP), np.float16)
        vTc[:, :ne] = value[b].T[:, idx].astype(np.float16)
        mvec = np.zeros((SP,), np.float16)
        mvec[:ne] = 1.0
        in_maps.append({
            "qT": np.ascontiguousarray(query[b].T.astype(np.float16)),
            "kT": kTc,
            "vT": vTc,
            "mv": mvec,
            "wqT": np.ascontiguousarray(Wq[hs:hs + HWID, :].T.astype(np.float16)),
            "wkT": np.ascontiguousarray(Wk[hs:hs + HWID, :].T.astype(np.float16)),
            "wvT": np.ascontiguousarray(Wv[hs:hs + HWID, :].T.astype(np.float16)),
            "woT": np.ascontiguousarray(Wo[:, hs:hs + HWID].T.astype(np.float16)),
        })

    res = run_bass_kernel_spmd(nc, in_maps, list(range(8))).results
    out = np.zeros((B, N, D), np.float32)
    for b in range(B):
        out[b] = res[4 * b]["out"] + res[4 * b + 1]["out"] \
            + res[4 * b + 2]["out"] + res[4 * b + 3]["out"] + bo
    return out


# revision 13
# speedup vs baseline: 12715.4172x; 1.0640x over previous
"""Cross-attention kernel for 8 Trainium2 NeuronCores.

Sharding: core c => batch b = c//4, head-group g = c%4 (3 of 12 heads, 192 dims).
Each core projects q/k/v for its heads, does softmax attention, and computes a
partial output projection (row-split Wo); host sums the 4 partials per batch.

Key tricks:
  - mask compaction on host: only mask==1 key/value positions are shipped
    (~2048 of 4096), zero-padded to a multiple of 128. Padded rows have
    zeroed v and zeroed ones-column so they contribute 0 to both numerator
    and denominator => exact equivalence with the reference's -1e4 bias.
  - transposed layouts end to end (contraction dim on partitions): no
    on-device transposes.
  - softmax without max-subtraction (scores*scale ~ N(0,1): exp safe in
    fp32) and without dividing the SxN score matrix: a ones-column appended
    to v yields the denominator Z per output row; only the 64xN attention
    output is normalized.
  - fp16 operands for all matmuls (fp32 PSUM accumulate).
  - q/k/v projections are spread through the attention j-loops as PE filler
    so the PE stream stays dense (HAM clock gate at 8/8), with emission
    software-pipelined (scores j+1 issued before attn j).
"""

import numpy as np

import concourse.bass as bass
import concourse.mybir as mybir
import concourse.tile as tile
from concourse import bacc
from concourse.bass_utils import run_bass_kernel_spmd

H = 12
D = 768
HD = 64
SCALE = HD ** -0.5
NQ = 1024
HL = 3            # heads per core
HWID = HL * HD    # 192 head dims per core
DC = D // 128     # 6 contraction chunks

f16 = mybir.dt.float16
f32 = mybir.dt.float32

_programs = {}


def _build(SP: int):
    NCH = SP // 128
    nc = bacc.Bacc("TRN2", target_bir_lowering=False, debug=False, num_devices=8)

    qT = nc.dram_tensor("qT", [D, NQ], f16, kind="ExternalInput")
    kT = nc.dram_tensor("kT", [D, SP], f16, kind="ExternalInput")
    vT = nc.dram_tensor("vT", [D, SP], f16, kind="ExternalInput")
    mv = nc.dram_tensor("mv", [SP], f16, kind="ExternalInput")
    wqT = nc.dram_tensor("wqT", [D, HWID], f16, kind="ExternalInput")
    wkT = nc.dram_tensor("wkT", [D, HWID], f16, kind="ExternalInput")
    wvT = nc.dram_tensor("wvT", [D, HWID], f16, kind="ExternalInput")
    woT = nc.dram_tensor("woT", [HWID, D], f16, kind="ExternalInput")
    out = nc.dram_tensor("out", [NQ, D], f32, kind="ExternalOutput")

    EXPF = mybir.ActivationFunctionType.Exp
    qT_r = qT.ap().rearrange("(c p) n -> p c n", p=128)
    kT_r = kT.ap().rearrange("(c p) n -> p c n", p=128)
    vT_r = vT.ap().rearrange("(c p) n -> p c n", p=128)

    with tile.TileContext(nc) as tc:
        with (
            tc.tile_pool(name="const", bufs=1) as cpool,
            tc.tile_pool(name="work", bufs=2) as wpool,
            tc.tile_pool(name="expp", bufs=14) as epool,
            tc.tile_pool(name="ps", bufs=2, space="PSUM") as pspool,
            tc.tile_pool(name="psf", bufs=2, space="PSUM") as psfpool,
            tc.tile_pool(name="psa", bufs=1, space="PSUM") as psapool,
        ):
            # ---- input DMAs, chunked + ordered so compute starts early
            wq_in = cpool.tile([128, DC, HWID], f16)
            nc.sync.dma_start(wq_in[:], wqT.ap().rearrange("(c p) n -> p c n", p=128))
            wk_in = cpool.tile([128, DC, HWID], f16)
            nc.sync.dma_start(wk_in[:], wkT.ap().rearrange("(c p) n -> p c n", p=128))
            SPA = min(1024, SP)          # first column block of kT/vT
            qT_in = cpool.tile([128, DC, NQ], f16)
            for d in range(DC):
                nc.sync.dma_start(qT_in[:, d, :], qT_r[:, d, :])
            kT_in = cpool.tile([128, DC, SP], f16)
            for d in range(DC):
                nc.sync.dma_start(kT_in[:, d, 0:SPA], kT_r[:, d, 0:SPA])
            wv_in = cpool.tile([128, DC, HWID], f16)
            nc.sync.dma_start(wv_in[:], wvT.ap().rearrange("(c p) n -> p c n", p=128))
            vT_in = cpool.tile([128, DC, SP], f16)
            for d in range(DC):
                nc.sync.dma_start(vT_in[:, d, 0:SPA], vT_r[:, d, 0:SPA])
            if SP > SPA:
                for d in range(DC):
                    nc.sync.dma_start(kT_in[:, d, SPA:SP], kT_r[:, d, SPA:SP])
                for d in range(DC):
                    nc.sync.dma_start(vT_in[:, d, SPA:SP], vT_r[:, d, SPA:SP])
            wo_in = cpool.tile([128, 2, D], f16)
            nc.sync.dma_start(wo_in[:, 0, :], woT[0:128, :])
            nc.sync.dma_start(wo_in[0:64, 1, :], woT[128:HWID, :])
            msk = cpool.tile([128, NCH], f16)
            nc.sync.dma_start(msk[:], mv.ap().rearrange("(c p) -> p c", p=128))

            q0 = cpool.tile([128, NQ], f16)
            q1 = cpool.tile([64, NQ], f16)
            k0 = cpool.tile([128, SP], f16)
            k1 = cpool.tile([64, SP], f16)
            vaug = cpool.tile([128, HL * NCH * 65], f16)
            vaug_r = vaug[:].rearrange("p (h j e) -> p h j e", h=HL, j=NCH)
            a0 = cpool.tile([128, NQ], f16)
            a1 = cpool.tile([64, NQ], f16)

            def proj_qk(w_in, src, dst, mt, sg, sw, pool, tag):
                mw = 128 if mt == 0 else 64
                ps = pool.tile([mw, sw], f32, tag=tag)
                for d in range(DC):
                    for nf in range(0, sw, 512):
                        wf = min(512, sw - nf)
                        nc.tensor.matmul(
                            ps[:, nf:nf + wf],
                            w_in[:, d, mt * 128:mt * 128 + mw],
                            src[:, d, sg + nf:sg + nf + wf],
                            start=(d == 0), stop=(d == DC - 1),
                        )
                nc.vector.tensor_copy(dst[:, sg:sg + sw], ps[:])

            def proj_v(j):
                ps = psfpool.tile([128, HWID], f32, tag="fill")
                for d in range(DC):
                    nc.tensor.matmul(
                        ps[:], vT_in[:, d, j * 128:(j + 1) * 128], wv_in[:, d, :],
                        start=(d == 0), stop=(d == DC - 1),
                    )
                nc.vector.tensor_copy(
                    vaug_r[:, :, j, 0:64], ps[:].rearrange("p (h e) -> p h e", h=HL)
                )

            def wo_mms(po, nt, kk):
                asrc, kw = ((a0, 128), (a1, 64))[kk]
                for nf in range(0, D, 512):
                    wf = min(512, D - nf)
                    nc.tensor.matmul(
                        po[:, nf:nf + wf],
                        asrc[:, nt * 128:(nt + 1) * 128],
                        wo_in[0:kw, kk, nf:nf + wf],
                        start=(kk == 0), stop=(kk == 1),
                    )

            LNF = mybir.ActivationFunctionType.Ln

            def normalize(at, adst):
                # 1/Z = exp(-ln Z), on ScalarE (keeps the DVE queue clear;
                # DVE's iterative reciprocal on [1, N] costs ~6.5us)
                lz = wpool.tile([1, NQ], f32, tag="lz")
                nc.scalar.activation(lz[:], at[64:65, :], LNF)
                rz = wpool.tile([1, NQ], f32, tag="rz")
                nc.scalar.activation(rz[:], lz[:], EXPF, scale=-1.0)
                rzb = wpool.tile([64, NQ], f32, tag="rzb")
                nc.gpsimd.partition_broadcast(rzb[:], rz[:])
                nc.vector.tensor_mul(adst, at[0:64, :], rzb[:])

            # mask column of vaug (depends only on msk DMA)
            nc.vector.tensor_copy(
                vaug_r[:, :, :, 64],
                msk[:].rearrange("p (u j) -> p u j", u=1).broadcast_to([128, HL, NCH]),
            )

            # ---- dense prologue: q/k projections for heads h0/h1 (m-tile 0)
            for sg in range(0, NQ, 1024):
                proj_qk(wq_in, qT_in, q0, 0, sg, min(1024, NQ - sg), pspool, "ps")
            for sg in range(0, SP, 1024):
                proj_qk(wk_in, kT_in, k0, 0, sg, min(1024, SP - sg), pspool, "ps")

            # m-tile-1 (head h2) projections: 512-wide chunks used as filler
            m1_work = []
            for sg in range(0, NQ, 512):
                m1_work.append((wq_in, qT_in, q1, sg, min(512, NQ - sg)))
            for sg in range(0, SP, 512):
                m1_work.append((wk_in, kT_in, k1, sg, min(512, SP - sg)))

            # ---- attention per head (software-pipelined emission)
            heads = (
                (k0[0:64, :], q0[0:64, :], a0[0:64, :]),
                (k0[64:128, :], q0[64:128, :], a0[64:128, :]),
                (k1[:, :], q1[:, :], a1[:, :]),
            )
            pending_norm = None
            for h in range(HL):
                kh, qh, adst = heads[h]
                at = psapool.tile([65, NQ], f32)
                prev_ex = None
                for j in range(NCH):
                    # PE filler to keep the matmul stream dense
                    if h == 0:
                        proj_v(j)
                        if j % 3 == 2 and m1_work:
                            w_in, src, dst, sg, sw = m1_work.pop(0)
                            proj_qk(w_in, src, dst, 1, sg, sw, psfpool, "fill")
                    elif h == 1 and j % 2 == 1 and m1_work:
                        w_in, src, dst, sg, sw = m1_work.pop(0)
                        proj_qk(w_in, src, dst, 1, sg, sw, psfpool, "fill")
                    sc = pspool.tile([128, NQ], f32, tag="ps")
                    for nf in range(0, NQ, 512):
                        nc.tensor.matmul(
                            sc[:, nf:nf + 512], kh[:, j * 128:(j + 1) * 128],
                            qh[:, nf:nf + 512], start=True, stop=True,
                        )
                    ex = epool.tile([128, NQ], f16)
                    nc.scalar.activation(ex[:], sc[:], EXPF, scale=SCALE)
                    if prev_ex is not None:
                        pj, pex = prev_ex
                        for nf in range(0, NQ, 512):
                            nc.tensor.matmul(
                                at[:, nf:nf + 512],
                                vaug[:, (h * NCH + pj) * 65:(h * NCH + pj) * 65 + 65],
                                pex[:, nf:nf + 512],
                                start=(pj == 0), stop=False,
                            )
                    prev_ex = (j, ex)
                    # previous head's normalize, emitted late so its DVE work
                    # lands behind this head's first filler copies
                    if j == 1 and pending_norm is not None:
                        normalize(*pending_norm)
                        pending_norm = None
                    if h == 1 and j == NCH - 1:
                        while m1_work:
                            w_in, src, dst, sg, sw = m1_work.pop(0)
                            proj_qk(w_in, src, dst, 1, sg, sw, psfpool, "fill")
                pj, pex = prev_ex
                for nf in range(0, NQ, 512):
                    nc.tensor.matmul(
                        at[:, nf:nf + 512],
                        vaug[:, (h * NCH + pj) * 65:(h * NCH + pj) * 65 + 65],
                        pex[:, nf:nf + 512],
                        start=(pj == 0), stop=True,
                    )
                pending_norm = (at, adst)

            # ---- output projection, overlapped with the last normalize
            po_prev = None
            po0 = pspool.tile([128, D], f32, tag="ps")
            wo_mms(po0, 0, 0)
            po1 = pspool.tile([128, D], f32, tag="ps")
            wo_mms(po1, 1, 0)
            normalize(*pending_norm)     # h2's normalize (gates only kk=1)
            pend = [(po0, 0), (po1, 1)]
            for nt in range(2, NQ // 128 + 2):
                po, pnt = pend.pop(0)
                wo_mms(po, pnt, 1)
                ob = wpool.tile([128, D], f32, tag="ob")
                nc.vector.tensor_copy(ob[:], po[:])
                nc.sync.dma_start(out[pnt * 128:(pnt + 1) * 128, :], ob[:])
                if nt < NQ // 128:
                    pon = pspool.tile([128, D], f32, tag="ps")
                    wo_mms(pon, nt, 0)
                    pend.append((pon, nt))

    nc.compile()
    return nc


def _get_program(SP: int):
    if SP not in _programs:
        _programs[SP] = _build(SP)
    return _programs[SP]


def kernel(query, key, value, mask, Wq, Wk, Wv, Wo, bo):
    query = np.asarray(query, np.float32)
    key = np.asarray(key, np.float32)
    value = np.asarray(value, np.float32)
    mask = np.asarray(mask, np.float32)
    Wq = np.asarray(Wq, np.float32)
    Wk = np.asarray(Wk, np.float32)
    Wv = np.asarray(Wv, np.float32)
    Wo = np.asarray(Wo, np.float32)
    bo = np.asarray(bo, np.float32)

    B, N, _ = query.shape
    idxs = [np.nonzero(mask[b] > 0.5)[0] for b in range(B)]
    se_max = max(len(i) for i in idxs)
    SP = max(((se_max + 127) // 128) * 128, 128)
    nc = _get_program(SP)

    in_maps = []
    for c in range(8):
        b, g = c // 4, c % 4
        hs = g * HWID
        idx = idxs[b]
        ne = len(idx)
        kTc = np.zeros((D, SP), np.float16)
        kTc[:, :ne] = key[b].T[:, idx].astype(np.float16)
        vTc = np.zeros((D, SP), np.float16)
        vTc[:, :ne] = value[b].T[:, idx].astype(np.float16)
        mvec = np.zeros((SP,), np.float16)
        mvec[:ne] = 1.0
        in_maps.append({
            "qT": np.ascontiguousarray(query[b].T.astype(np.float16)),
            "kT": kTc,
            "vT": vTc,
            "mv": mvec,
            "wqT": np.ascontiguousarray(Wq[hs:hs + HWID, :].T.astype(np.float16)),
            "wkT": np.ascontiguousarray(Wk[hs:hs + HWID, :].T.astype(np.float16)),
            "wvT": np.ascontiguousarray(Wv[hs:hs + HWID, :].T.astype(np.float16)),
            "woT": np.ascontiguousarray(Wo[:, hs:hs + HWID].T.astype(np.float16)),
        })

    res = run_bass_kernel_spmd(nc, in_maps, list(range(8))).results
    out = np.zeros((B, N, D), np.float32)
    for b in range(B):
        out[b] = res[4 * b]["out"] + res[4 * b + 1]["out"] \
            + res[4 * b + 2]["out"] + res[4 * b + 3]["out"] + bo
    return out
